# revision 6
# baseline (speedup 1.0000x reference)
"""Trainium2 Bass kernel for nn_BinaryDiceLoss_blobPunish.

Reference semantics (B=16, H=W=512):
    thr   = predict.max() / 2
    mask  = predict > thr
    labels = 200 iterations of masked 3x3 max-pool label propagation
    n_unique = number of distinct label values
    penalty = n_unique/B; if penalty < 1 -> B; min(penalty, B)
    dice_i = 1 - (sum(p_i*t_i)+1) / (sum(p_i^2)+sum(t_i^2)+1)
    out = mean(dice_i) * penalty

Distribution: data-parallel over batch, 2 images per core on 8 cores.

Device work (two SPMD launches):
  Launch A: per-image dice partial sums (sum p*t, sum p^2, sum t^2) and
            per-core max of predict.      Reads p+t (4MB/core).
  Launch B: given thr, count isolated mask pixels (mask pixel whose 8
            in-image neighbors are all background).  Reads p (2MB/core).

Penalty shortcut (exact): every isolated mask pixel keeps its own unique
label forever under 3x3 max-pool propagation, and those labels are
distinct values of the final label tensor.  At most one of them (global
pixel id 0) can collide with the background value 0.  So
n_unique >= n_isolated - 1.  If n_isolated >= 257 then n_unique >= 256,
hence penalty == min(n_unique/16, 16) == 16 exactly.  For the uniform
random inputs this problem generates, n_isolated ~ 8000, so the device
path decides the penalty; a full numpy fallback covers other inputs.

Isolated-pixel counting on device:
  m     = (p > thr)                     (bf16 0/1, zero-padded borders)
  H1    = m[r,c-1] + m[r,c+1]           (horizontal neighbors)
  psum  = T3 @ H1 + C @ m (+ U/L cross-chunk row corrections)
        = (3x3 sum of m) - 2*m          (via C = T3 - 2I)
  iso   = (psum == -1)                  (mask pixel with 3x3 sum == 1)
"""

from contextlib import ExitStack

import numpy as np

B = 16
H = 512
W = 512
N_CORES = 8
IPC = B // N_CORES  # images per core
RPC = IPC * H  # rows per core (shard shape [RPC, W])
NCHUNK = H // 128  # 128-row chunks per image

_cache: dict = {}


def _install_ntff_hook():
    """Make trace=True work under axon: the stub antenv package lacks
    axon_hooks, so boot() silently skipped NTFF hook registration."""
    import sys
    import types

    if "antenv.axon_hooks" in sys.modules:
        return
    try:
        import antenv

        mod = types.ModuleType("antenv.axon_hooks")
        mod._hook = None
        mod.set_axon_ntff_profile_hook = lambda h: setattr(mod, "_hook", h)
        mod.get_axon_ntff_profile_hook = lambda: mod._hook
        sys.modules["antenv.axon_hooks"] = mod
        antenv.axon_hooks = mod
        from trn_agent_boot.trn_boot import _ntff_profile_via_ctypes

        hook = _ntff_profile_via_ctypes("/opt/axon/libaxon_pjrt.so")
        if hook is not None:
            mod.set_axon_ntff_profile_hook(hook)
    except Exception:
        pass


def _build_launch_a():
    import concourse.bacc as bacc
    import concourse.tile as tile
    from concourse import mybir

    f32 = mybir.dt.float32
    A = mybir.AluOpType
    nc = bacc.Bacc("TRN2", target_bir_lowering=False, debug=False, num_devices=N_CORES)
    p = nc.dram_tensor("p", [RPC, W], f32, kind="ExternalInput").ap()
    t = nc.dram_tensor("t", [RPC, W], f32, kind="ExternalInput").ap()
    # acc cols: 0:8 sum(p*t) per chunk, 8:16 max(p) per chunk,
    #           16:24 sum(p^2), 24:32 sum(t^2)
    acc = nc.dram_tensor("acc", [128, 32], f32, kind="ExternalOutput").ap()

    p_v = p.rearrange("(n q) m -> n q m", q=128)  # [8, 128, 512]
    t_v = t.rearrange("(n q) m -> n q m", q=128)

    with tile.TileContext(nc) as tc:
        with ExitStack() as ctx:
            loadp = ctx.enter_context(tc.tile_pool(name="loadp", bufs=3))
            loadt = ctx.enter_context(tc.tile_pool(name="loadt", bufs=3))
            scr = ctx.enter_context(tc.tile_pool(name="scr", bufs=2))
            accp = ctx.enter_context(tc.tile_pool(name="accp", bufs=1))

            dve_acc = accp.tile([128, 16], f32)  # pt 0:8, mx 8:16
            act_acc = accp.tile([128, 16], f32)  # p2 0:8, t2 8:16

            for j in range(IPC * NCHUNK):
                pt_ = loadp.tile([128, W], f32, tag="p")
                nc.sync.dma_start(pt_[:], p_v[j])
                tt_ = loadt.tile([128, W], f32, tag="t")
                nc.sync.dma_start(tt_[:], t_v[j])

                s0 = scr.tile([128, W], f32, tag="s0")
                nc.gpsimd.tensor_mul(s0[:], pt_[:], tt_[:])
                nc.vector.reduce_sum(
                    dve_acc[:, j : j + 1], s0[:], axis=mybir.AxisListType.X
                )
                nc.vector.reduce_max(
                    dve_acc[:, 8 + j : 9 + j], pt_[:], axis=mybir.AxisListType.X
                )
                s1 = scr.tile([128, W], mybir.dt.bfloat16, tag="s1")
                nc.scalar.activation(
                    s1[:],
                    pt_[:],
                    mybir.ActivationFunctionType.Square,
                    accum_out=act_acc[:, j : j + 1],
                )
                s2 = scr.tile([128, W], mybir.dt.bfloat16, tag="s2")
                nc.scalar.activation(
                    s2[:],
                    tt_[:],
                    mybir.ActivationFunctionType.Square,
                    accum_out=act_acc[:, 8 + j : 9 + j],
                )

            nc.sync.dma_start(acc[:, 0:16], dve_acc[:])
            nc.sync.dma_start(acc[:, 16:32], act_acc[:])
    nc.compile()
    return nc


def _build_launch_b():
    import concourse.bacc as bacc
    import concourse.tile as tile
    from concourse import mybir

    f32 = mybir.dt.float32
    bf16 = mybir.dt.bfloat16
    A = mybir.AluOpType
    nc = bacc.Bacc("TRN2", target_bir_lowering=False, debug=False, num_devices=N_CORES)
    p = nc.dram_tensor("p", [RPC, W], f32, kind="ExternalInput").ap()
    thr = nc.dram_tensor("thr", [128, 1], f32, kind="ExternalInput").ap()
    # tri cols: 0:128 T3 (tridiag ones), 128:256 C = T3 - 2I,
    #           256:384 U (lhsT[127,0]=1), 384:512 L (lhsT[0,127]=1)
    tri = nc.dram_tensor("tri", [128, 4 * 128], bf16, kind="ExternalInput").ap()
    iso = nc.dram_tensor("iso", [128, NCHUNK], f32, kind="ExternalOutput").ap()

    # [128, img, chunk, col]: row of shard = img*512 + chunk*128 + q
    p_v = p.rearrange("(i n q) m -> q i n m", i=IPC, q=128)

    with tile.TileContext(nc) as tc:
        with ExitStack() as ctx:
            consts = ctx.enter_context(tc.tile_pool(name="consts", bufs=1))
            loadp = ctx.enter_context(tc.tile_pool(name="loadp", bufs=4))
            maskp = ctx.enter_context(tc.tile_pool(name="maskp", bufs=NCHUNK))
            h1p = ctx.enter_context(tc.tile_pool(name="h1p", bufs=NCHUNK))
            scr = ctx.enter_context(tc.tile_pool(name="scr", bufs=2))
            accp = ctx.enter_context(tc.tile_pool(name="accp", bufs=1))
            psump = ctx.enter_context(tc.tile_pool(name="psump", bufs=2, space="PSUM"))

            tri_t = consts.tile([128, 4 * 128], bf16)
            nc.sync.dma_start(tri_t[:], tri[:])
            thr_t = consts.tile([128, 1], f32)
            nc.sync.dma_start(thr_t[:], thr[:])
            iso_acc = accp.tile([128, NCHUNK], f32)

            masks = []
            h1s = []
            for k in range(NCHUNK):
                p_ = loadp.tile([128, IPC, W], f32, tag="p")
                nc.sync.dma_start(p_[:], p_v[:, :, k, :])

                # mask block per image: [border, 512 cols, border] = 514
                m_ = maskp.tile([128, IPC, W + 2], bf16, tag="m")
                nc.vector.memset(m_[:, :, 0 : W + 2 : W + 1], 0.0)
                nc.vector.tensor_scalar(
                    m_[:, :, 1 : W + 1], p_[:], thr_t[:], None, A.is_gt
                )
                h_ = h1p.tile([128, IPC, W], bf16, tag="h")
                nc.gpsimd.tensor_add(h_[:], m_[:, :, 0:W], m_[:, :, 2 : W + 2])
                masks.append(m_)
                h1s.append(h_)

            for k in range(NCHUNK):
                ps = psump.tile([128, IPC * W], f32, tag="ps")
                for i in range(IPC):
                    dst = ps[:, i * W : (i + 1) * W]
                    mms = [
                        (tri_t[:, 0:128], h1s[k][:, i, :]),
                        (tri_t[:, 128:256], masks[k][:, i, 1 : W + 1]),
                    ]
                    if k > 0:
                        mms.append((tri_t[:, 256:384], h1s[k - 1][:, i, :]))
                        mms.append((tri_t[:, 256:384], masks[k - 1][:, i, 1 : W + 1]))
                    if k < NCHUNK - 1:
                        mms.append((tri_t[:, 384:512], h1s[k + 1][:, i, :]))
                        mms.append((tri_t[:, 384:512], masks[k + 1][:, i, 1 : W + 1]))
                    for q, (lhsT, rhs) in enumerate(mms):
                        nc.tensor.matmul(
                            dst,
                            lhsT,
                            rhs,
                            start=(q == 0),
                            stop=(q == len(mms) - 1),
                        )
                # iso indicator of (ps == -1) for integer-valued ps in [-2, 9]:
                # sq = (ps+1)^2, then relu(1-sq) is 1 iff ps == -1, else 0.
                sq = scr.tile([128, IPC * W], bf16, tag="sq")
                nc.scalar.activation(
                    sq[:],
                    ps[:],
                    mybir.ActivationFunctionType.Square,
                    bias=1.0,
                    scale=1.0,
                )
                so = scr.tile([128, IPC * W], bf16, tag="so")
                nc.scalar.activation(
                    so[:],
                    sq[:],
                    mybir.ActivationFunctionType.Relu,
                    bias=1.0,
                    scale=-1.0,
                    accum_out=iso_acc[:, k : k + 1],
                )

            nc.sync.dma_start(iso[:], iso_acc[:])
    nc.compile()
    return nc


def _tri_matrices():
    import ml_dtypes

    tri = np.zeros((128, 4 * 128), np.float32)
    idx = np.arange(128)
    T3 = tri[:, 0:128]
    T3[idx, idx] = 1.0
    T3[idx[:-1], idx[:-1] + 1] = 1.0
    T3[idx[:-1] + 1, idx[:-1]] = 1.0
    C = tri[:, 128:256]
    C[:] = T3
    C[idx, idx] = -1.0
    tri[127, 256 + 0] = 1.0  # U
    tri[0, 384 + 127] = 1.0  # L
    return tri.astype(ml_dtypes.bfloat16)


def _penalty_fallback(predict):
    """Exact numpy replica of the reference penalty path (rarely used)."""
    p = np.asarray(predict, np.float32).reshape(B, H, W)
    thr = np.float32(p.max()) / np.float32(2.0)
    mask = p > thr
    init = np.arange(B * H * W, dtype=np.float32).reshape(B, H, W)
    lab = np.where(mask, init, np.float32(0.0))
    pad = np.empty((B, H + 2, W + 2), np.float32)
    for _ in range(200):
        pad.fill(-np.inf)
        pad[:, 1:-1, 1:-1] = lab
        mx = pad[:, 0:-2, 0:-2]
        for dr in range(3):
            for dc in range(3):
                if dr == 0 and dc == 0:
                    continue
                mx = np.maximum(mx, pad[:, dr : dr + H, dc : dc + W])
        new = np.where(mask, mx, np.float32(0.0))
        if np.array_equal(new, lab):
            lab = new
            break
        lab = new
    n_unique = np.unique(lab).size
    penalty = np.float32(n_unique) / np.float32(B)
    if penalty < 1.0:
        penalty = np.float32(B)
    return float(min(penalty, np.float32(B)))


def _get_built():
    if "a" not in _cache:
        _cache["a"] = _build_launch_a()
    if "b" not in _cache:
        _cache["b"] = _build_launch_b()
    return _cache["a"], _cache["b"]


LAST_PERF: dict = {}


def kernel(predict, target):
    import os

    from concourse.bass_utils import run_bass_kernel_spmd

    trace = bool(os.environ.get("BDICE_TRACE"))
    if trace:
        _install_ntff_hook()

    pred = np.ascontiguousarray(np.asarray(predict, np.float32).reshape(B * H, W))
    targ = np.ascontiguousarray(np.asarray(target, np.float32).reshape(B * H, W))
    p_sh = pred.reshape(N_CORES, RPC, W)
    t_sh = targ.reshape(N_CORES, RPC, W)

    nc_a, nc_b = _get_built()
    core_ids = list(range(N_CORES))

    in_a = [{"p": p_sh[c], "t": t_sh[c]} for c in range(N_CORES)]
    res_a = run_bass_kernel_spmd(nc_a, in_a, core_ids=core_ids, trace=trace)

    acc = np.stack([res_a.results[c]["acc"] for c in range(N_CORES)])  # [8,128,32]
    gmax = np.float32(acc[:, :, 8:16].max())
    thr = gmax / np.float32(2.0)

    tri = _tri_matrices()
    thr_arr = np.full((128, 1), thr, np.float32)
    in_b = [{"p": p_sh[c], "thr": thr_arr, "tri": tri} for c in range(N_CORES)]
    res_b = run_bass_kernel_spmd(nc_b, in_b, core_ids=core_ids, trace=trace)
    if trace:
        LAST_PERF.update(
            a_ns=res_a.exec_time_ns,
            b_ns=res_b.exec_time_ns,
            a_trace=(res_a.instructions_and_trace or (None, None))[1],
            b_trace=(res_b.instructions_and_trace or (None, None))[1],
        )
    iso_total = int(
        round(float(sum(res_b.results[c]["iso"].sum(dtype=np.float64) for c in core_ids)))
    )

    if iso_total >= 257:
        penalty = 16.0
    else:
        penalty = _penalty_fallback(pred)

    acc64 = acc.astype(np.float64)
    losses = []
    for c in range(N_CORES):
        for i in range(IPC):
            pt = acc64[c, :, i * 4 : (i + 1) * 4].sum()
            p2 = acc64[c, :, 16 + i * 4 : 16 + (i + 1) * 4].sum()
            t2 = acc64[c, :, 24 + i * 4 : 24 + (i + 1) * 4].sum()
            losses.append(1.0 - (pt + 1.0) / (p2 + t2 + 1.0))
    mean_loss = float(np.mean(losses))
    return np.float32(mean_loss * penalty)
